# revision 2
# baseline (speedup 1.0000x reference)
"""Trainium2 Bass kernel for nn_Advection (2D advection stencil).

    out[b,i,j] = (s[b,i+1,j]-s[b,i,j])*v[b,i,j,0]
               + (s[b,i,j+1]-s[b,i,j])*v[b,i,j,1]
with symmetric edge padding (forward difference is 0 on the last row/col).

Sharding: pure data parallel — batch 32 split 4-per-core across 8 NeuronCores.

Per-core implementation (memory-bound; steady state measured at ~343 GB/s/core
of HBM traffic ≈ 96% of the per-core ceiling):
 - Stripe layout: each 512x512 image lives in SBUF as [128 partitions, 2048];
   partition p, free block k (of 4) holds image row k*128 + p. DRAM reads are
   2KB-contiguous per (partition, block) — full DMA line rate.
 - dy (row shift) runs on the TensorEngine as a banded-difference matmul:
   dy_stripe = D^T @ s_stripe with D = subdiag(+1) + diag(-1), accumulated in
   PSUM; a second K=1 matmul with E (e_0 -> row 127) adds the next stripe's
   first row; the last stripe uses D3 (D with the corner zeroed) so row 511's
   dy is exactly 0. Matmuls use float32r (TF32-like, 1 cycle/row; ~1.3e-4
   relative error) so the PE is never the critical path even when HAM-cold.
 - dx (col shift) is a free-dim shifted subtract on the VectorEngine; the
   j=511 junk columns are zeroed by one strided memset.
 - Products and the final add run on the VectorEngine, in-place to save SBUF.
 - DMA rings are kept load/store-pure to avoid HWDGE FIFO head-of-line
   blocking: state loads on the gpsimd SWDGE ring (f32->float32r cast during
   DMA), velocity loads on the scalar HWDGE ring, output stores on the sync
   HWDGE ring. 2-image groups give 2-4MB transfers (DMA efficiency) while
   double-buffered pools overlap load/compute/store.
"""

import numpy as np

B, H, W = 32, 512, 512
N_CORES = 8
B_PER = B // N_CORES   # 4 images per core
P = 128                # SBUF partitions
KS = H // P            # 4 stripes per image
FD = KS * W            # 2048 free elems per partition per image
G = 2                  # image groups per core
IPG = B_PER // G       # images per group

_cache = {}


def _consts():
    D = np.zeros((P, P), np.float32)
    for m in range(P):
        D[m, m] = -1.0
        if m + 1 < P:
            D[m + 1, m] = 1.0
    D3 = D.copy()
    D3[P - 1, P - 1] = 0.0
    E = np.zeros((1, P), np.float32)
    E[0, P - 1] = 1.0
    return {"dmat": D, "dmat3": D3, "emat": E}


def build_nc(repeats=1):
    """Build + compile the per-core program. repeats>1 wraps the body in an
    on-device loop (benchmarking only; production uses repeats=1)."""
    from contextlib import ExitStack

    import concourse.tile as tile
    from concourse import bacc, mybir

    f32 = mybir.dt.float32
    f32r = mybir.dt.float32r

    nc = bacc.Bacc("TRN2", target_bir_lowering=False)
    state = nc.dram_tensor("state", [B_PER, H, W, 1], f32, kind="ExternalInput")
    vel = nc.dram_tensor("vel", [B_PER, H, W, 2], f32, kind="ExternalInput")
    out = nc.dram_tensor("out", [B_PER, H, W, 1], f32, kind="ExternalOutput")
    dmat = nc.dram_tensor("dmat", [P, P], f32, kind="ExternalInput")
    dmat3 = nc.dram_tensor("dmat3", [P, P], f32, kind="ExternalInput")
    emat = nc.dram_tensor("emat", [1, P], f32, kind="ExternalInput")

    # stripe-layout views: [img, partition, stripe, cols]
    state_v = state.ap().rearrange("b (k p) w c -> b p k (w c)", p=P)
    vel_v = vel.ap().rearrange("b (k p) w c -> b p k (w c)", p=P)
    out_v = out.ap().rearrange("(g b) (k p) w c -> g p b k (w c)", g=G, p=P)

    with tile.TileContext(nc) as tc:
        with ExitStack() as ctx:
            cp = ctx.enter_context(tc.tile_pool(name="cp", bufs=1))
            sp = ctx.enter_context(tc.tile_pool(name="sp", bufs=2))
            vp = ctx.enter_context(tc.tile_pool(name="vp", bufs=2))
            xp = ctx.enter_context(tc.tile_pool(name="xp", bufs=2))
            tp = ctx.enter_context(tc.tile_pool(name="tp", bufs=2))
            pp = ctx.enter_context(tc.tile_pool(name="pp", bufs=2, space="PSUM"))

            D = cp.tile([P, P], f32r)
            nc.gpsimd.dma_start(D[:], dmat.ap())
            D3 = cp.tile([P, P], f32r)
            nc.gpsimd.dma_start(D3[:], dmat3.ap())
            E = cp.tile([1, P], f32r)
            nc.gpsimd.dma_start(E[:], emat.ap())

            def body():
                for g in range(G):
                    s2 = sp.tile([P, IPG * FD], f32r, name=f"s2_{g}", tag="s2")
                    for b in range(IPG):
                        nc.gpsimd.dma_start(s2[:, b * FD:(b + 1) * FD],
                                            state_v[g * IPG + b])
                    v2t = vp.tile([P, IPG * 2 * FD], f32, name=f"v2_{g}",
                                  tag="v2")
                    for b in range(IPG):
                        nc.scalar.dma_start(v2t[:, 2 * b * FD:2 * (b + 1) * FD],
                                            vel_v[g * IPG + b])

                    dy_list = []
                    for b in range(IPG):
                        sb = s2[:, b * FD:(b + 1) * FD]
                        dy_ps = pp.tile([P, FD], f32, name=f"dy{g}{b}",
                                        tag="dy")
                        for k in range(3):
                            nc.tensor.matmul(dy_ps[:, k * W:(k + 1) * W], D[:],
                                             sb[:, k * W:(k + 1) * W],
                                             start=True, stop=False)
                        nc.tensor.matmul(dy_ps[:, 3 * W:4 * W], D3[:],
                                         sb[:, 3 * W:4 * W],
                                         start=True, stop=True)
                        for k in range(3):
                            nc.tensor.matmul(dy_ps[:, k * W:(k + 1) * W], E[:],
                                             sb[0:1, (k + 1) * W:(k + 2) * W],
                                             start=False, stop=True)
                        dy_list.append(dy_ps)

                    s2f = s2[:].bitcast(f32)
                    dx2 = xp.tile([P, IPG * FD], f32, name=f"dx2_{g}",
                                  tag="dx2")
                    nc.vector.tensor_sub(dx2[:, 0:IPG * FD - 1],
                                         s2f[:, 1:IPG * FD],
                                         s2f[:, 0:IPG * FD - 1])
                    nc.vector.memset(dx2[:, W - 1::W], 0.0)

                    t12 = tp.tile([P, IPG * FD], f32, name=f"t12_{g}",
                                  tag="t12")
                    for b in range(IPG):
                        nc.vector.tensor_mul(
                            t12[:, b * FD:(b + 1) * FD], dy_list[b][:],
                            v2t[:, 2 * b * FD:2 * (b + 1) * FD:2])
                    nc.vector.tensor_mul(dx2[:], dx2[:], v2t[:, 1::2])
                    nc.vector.tensor_add(t12[:], t12[:], dx2[:])
                    nc.sync.dma_start(out_v[g], t12[:])

            if repeats > 1:
                with tc.For_i(0, repeats) as _:
                    body()
            else:
                body()

    nc.compile()
    return nc


def _get_nc():
    if "nc" not in _cache:
        _cache["nc"] = build_nc()
    return _cache["nc"]


def kernel(state_variable: np.ndarray, velocity_field: np.ndarray) -> np.ndarray:
    from concourse.bass_utils import run_bass_kernel_spmd

    nc = _get_nc()
    state_variable = np.ascontiguousarray(state_variable, dtype=np.float32)
    velocity_field = np.ascontiguousarray(velocity_field, dtype=np.float32)
    consts = _consts()
    in_maps = []
    for c in range(N_CORES):
        lo, hi = c * B_PER, (c + 1) * B_PER
        in_maps.append({
            "state": state_variable[lo:hi],
            "vel": velocity_field[lo:hi],
            **consts,
        })
    res = run_bass_kernel_spmd(nc, in_maps, core_ids=list(range(N_CORES)))
    return np.concatenate([r["out"] for r in res.results], axis=0)


# revision 3
# speedup vs baseline: 1.1083x; 1.1083x over previous
"""Trainium2 Bass kernel for nn_Advection (2D advection stencil).

    out[b,i,j] = (s[b,i+1,j]-s[b,i,j])*v[b,i,j,0]
               + (s[b,i,j+1]-s[b,i,j])*v[b,i,j,1]
with symmetric edge padding (forward difference is 0 on the last row/col).

Sharding: pure data parallel — batch 32 split 4-per-core across 8 NeuronCores.

Per-core implementation (memory-bound; steady state measured at ~343 GB/s/core
of HBM traffic ≈ 96% of the per-core ceiling):
 - Stripe layout: each 512x512 image lives in SBUF as [128 partitions, 2048];
   partition p, free block k (of 4) holds image row k*128 + p. DRAM reads are
   2KB-contiguous per (partition, block) — full DMA line rate.
 - dy (row shift) runs on the TensorEngine as a banded-difference matmul:
   dy_stripe = D^T @ s_stripe with D = subdiag(+1) + diag(-1), accumulated in
   PSUM; a second K=1 matmul with E (e_0 -> row 127) adds the next stripe's
   first row; the last stripe uses D3 (D with the corner zeroed) so row 511's
   dy is exactly 0. fp32 matmuls keep the result exact to ~5e-7 relative; the
   PE stays off the critical path (DMA-bound kernel).
 - dx (col shift) is a free-dim shifted subtract on the VectorEngine; the
   j=511 junk columns are zeroed by one strided memset.
 - Products and the final add run on the VectorEngine, in-place to save SBUF.
 - DMA rings are kept load/store-pure to avoid HWDGE FIFO head-of-line
   blocking: state loads on the gpsimd SWDGE ring, velocity loads on the scalar HWDGE ring, output stores on the sync
   HWDGE ring. 2-image groups give 2-4MB transfers (DMA efficiency) while
   double-buffered pools overlap load/compute/store.
"""

import numpy as np

B, H, W = 32, 512, 512
N_CORES = 8
B_PER = B // N_CORES   # 4 images per core
P = 128                # SBUF partitions
KS = H // P            # 4 stripes per image
FD = KS * W            # 2048 free elems per partition per image
G = 2                  # image groups per core
IPG = B_PER // G       # images per group

_cache = {}


def _consts():
    D = np.zeros((P, P), np.float32)
    for m in range(P):
        D[m, m] = -1.0
        if m + 1 < P:
            D[m + 1, m] = 1.0
    D3 = D.copy()
    D3[P - 1, P - 1] = 0.0
    E = np.zeros((1, P), np.float32)
    E[0, P - 1] = 1.0
    return {"dmat": D, "dmat3": D3, "emat": E}


def build_nc(repeats=1):
    """Build + compile the per-core program. repeats>1 wraps the body in an
    on-device loop (benchmarking only; production uses repeats=1)."""
    from contextlib import ExitStack

    import concourse.tile as tile
    from concourse import bacc, mybir

    f32 = mybir.dt.float32

    nc = bacc.Bacc("TRN2", target_bir_lowering=False)
    state = nc.dram_tensor("state", [B_PER, H, W, 1], f32, kind="ExternalInput")
    vel = nc.dram_tensor("vel", [B_PER, H, W, 2], f32, kind="ExternalInput")
    out = nc.dram_tensor("out", [B_PER, H, W, 1], f32, kind="ExternalOutput")
    dmat = nc.dram_tensor("dmat", [P, P], f32, kind="ExternalInput")
    dmat3 = nc.dram_tensor("dmat3", [P, P], f32, kind="ExternalInput")
    emat = nc.dram_tensor("emat", [1, P], f32, kind="ExternalInput")

    # stripe-layout views: [img, partition, stripe, cols]
    state_v = state.ap().rearrange("b (k p) w c -> b p k (w c)", p=P)
    vel_v = vel.ap().rearrange("b (k p) w c -> b p k (w c)", p=P)
    out_v = out.ap().rearrange("(g b) (k p) w c -> g p b k (w c)", g=G, p=P)

    with tile.TileContext(nc) as tc:
        with ExitStack() as ctx:
            cp = ctx.enter_context(tc.tile_pool(name="cp", bufs=1))
            sp = ctx.enter_context(tc.tile_pool(name="sp", bufs=2))
            vp = ctx.enter_context(tc.tile_pool(name="vp", bufs=2))
            xp = ctx.enter_context(tc.tile_pool(name="xp", bufs=2))
            tp = ctx.enter_context(tc.tile_pool(name="tp", bufs=2))
            pp = ctx.enter_context(tc.tile_pool(name="pp", bufs=2, space="PSUM"))

            D = cp.tile([P, P], f32)
            nc.gpsimd.dma_start(D[:], dmat.ap())
            D3 = cp.tile([P, P], f32)
            nc.gpsimd.dma_start(D3[:], dmat3.ap())
            E = cp.tile([1, P], f32)
            nc.gpsimd.dma_start(E[:], emat.ap())

            def body():
                for g in range(G):
                    s2 = sp.tile([P, IPG * FD], f32, name=f"s2_{g}", tag="s2")
                    for b in range(IPG):
                        nc.gpsimd.dma_start(s2[:, b * FD:(b + 1) * FD],
                                            state_v[g * IPG + b])
                    v2t = vp.tile([P, IPG * 2 * FD], f32, name=f"v2_{g}",
                                  tag="v2")
                    for b in range(IPG):
                        nc.scalar.dma_start(v2t[:, 2 * b * FD:2 * (b + 1) * FD],
                                            vel_v[g * IPG + b])

                    dy_list = []
                    for b in range(IPG):
                        sb = s2[:, b * FD:(b + 1) * FD]
                        dy_ps = pp.tile([P, FD], f32, name=f"dy{g}{b}",
                                        tag="dy")
                        for k in range(3):
                            nc.tensor.matmul(dy_ps[:, k * W:(k + 1) * W], D[:],
                                             sb[:, k * W:(k + 1) * W],
                                             start=True, stop=False)
                        nc.tensor.matmul(dy_ps[:, 3 * W:4 * W], D3[:],
                                         sb[:, 3 * W:4 * W],
                                         start=True, stop=True)
                        for k in range(3):
                            nc.tensor.matmul(dy_ps[:, k * W:(k + 1) * W], E[:],
                                             sb[0:1, (k + 1) * W:(k + 2) * W],
                                             start=False, stop=True)
                        dy_list.append(dy_ps)

                    s2f = s2
                    dx2 = xp.tile([P, IPG * FD], f32, name=f"dx2_{g}",
                                  tag="dx2")
                    nc.vector.tensor_sub(dx2[:, 0:IPG * FD - 1],
                                         s2f[:, 1:IPG * FD],
                                         s2f[:, 0:IPG * FD - 1])
                    nc.vector.memset(dx2[:, W - 1::W], 0.0)

                    t12 = tp.tile([P, IPG * FD], f32, name=f"t12_{g}",
                                  tag="t12")
                    for b in range(IPG):
                        nc.vector.tensor_mul(
                            t12[:, b * FD:(b + 1) * FD], dy_list[b][:],
                            v2t[:, 2 * b * FD:2 * (b + 1) * FD:2])
                    nc.vector.tensor_mul(dx2[:], dx2[:], v2t[:, 1::2])
                    nc.vector.tensor_add(t12[:], t12[:], dx2[:])
                    nc.sync.dma_start(out_v[g], t12[:])

            if repeats > 1:
                with tc.For_i(0, repeats) as _:
                    body()
            else:
                body()

    nc.compile()
    return nc


def _get_nc():
    if "nc" not in _cache:
        _cache["nc"] = build_nc()
    return _cache["nc"]


def kernel(state_variable: np.ndarray, velocity_field: np.ndarray) -> np.ndarray:
    from concourse.bass_utils import run_bass_kernel_spmd

    nc = _get_nc()
    state_variable = np.ascontiguousarray(state_variable, dtype=np.float32)
    velocity_field = np.ascontiguousarray(velocity_field, dtype=np.float32)
    consts = _consts()
    in_maps = []
    for c in range(N_CORES):
        lo, hi = c * B_PER, (c + 1) * B_PER
        in_maps.append({
            "state": state_variable[lo:hi],
            "vel": velocity_field[lo:hi],
            **consts,
        })
    res = run_bass_kernel_spmd(nc, in_maps, core_ids=list(range(N_CORES)))
    return np.concatenate([r["out"] for r in res.results], axis=0)


# revision 4
# speedup vs baseline: 1.1600x; 1.0467x over previous
"""Trainium2 Bass kernel for nn_Advection (2D advection stencil).

    out[b,i,j] = (s[b,i+1,j]-s[b,i,j])*v[b,i,j,0]
               + (s[b,i,j+1]-s[b,i,j])*v[b,i,j,1]
with symmetric edge padding (forward difference is 0 on the last row/col).

Sharding: pure data parallel — batch 32 split 4-per-core across 8 NeuronCores.

Per-core implementation (memory-bound; steady state measured at ~340 GB/s/core
of HBM traffic ≈ 95% of the ~358 GB/s per-core ceiling):
 - Stripe layout: each 512x512 image lives in SBUF as [128 partitions, 2048];
   partition p, free block k (of 4) holds image row k*128 + p. DRAM reads are
   2KB-contiguous per (partition, block) — full DMA line rate.
 - dy (row shift) runs on the TensorEngine as a banded-difference matmul:
   dy_stripe = D^T @ s_stripe with D = subdiag(+1) + diag(-1), accumulated in
   PSUM; a second K=1 matmul with E (e_0 -> row 127) adds the next stripe's
   first row; the last stripe uses D3 (D with the corner zeroed) so row 511's
   dy is exactly 0. fp32 matmuls keep the result exact to ~5e-8 relative and
   the PE stays off the critical path (DMA-bound kernel).
 - dx (col shift) is a free-dim shifted subtract on the VectorEngine; the
   j=511 junk columns are zeroed by one strided memset.
 - Products and the final add run on the VectorEngine, in-place to save SBUF.
 - DMA rings are kept load/store-pure to avoid HWDGE FIFO head-of-line
   blocking: state loads on the gpsimd SWDGE ring, velocity loads on the
   scalar HWDGE ring, output stores on the sync HWDGE ring. Per-image 1-2MB
   transfers with triple-buffered pools overlap load/compute/store (measured
   faster than coarser 2-image granularity at bufs=2).
"""

import numpy as np

B, H, W = 32, 512, 512
N_CORES = 8
B_PER = B // N_CORES   # 4 images per core
P = 128                # SBUF partitions
KS = H // P            # 4 stripes per image
FD = KS * W            # 2048 free elems per partition per image

_cache = {}


def _consts():
    D = np.zeros((P, P), np.float32)
    for m in range(P):
        D[m, m] = -1.0
        if m + 1 < P:
            D[m + 1, m] = 1.0
    D3 = D.copy()
    D3[P - 1, P - 1] = 0.0
    E = np.zeros((1, P), np.float32)
    E[0, P - 1] = 1.0
    return {"dmat": D, "dmat3": D3, "emat": E}


def build_nc(repeats=1):
    """Build + compile the per-core program. repeats>1 wraps the body in an
    on-device loop (benchmarking only; production uses repeats=1)."""
    from contextlib import ExitStack

    import concourse.tile as tile
    from concourse import bacc, mybir

    f32 = mybir.dt.float32

    nc = bacc.Bacc("TRN2", target_bir_lowering=False)
    state = nc.dram_tensor("state", [B_PER, H, W, 1], f32, kind="ExternalInput")
    vel = nc.dram_tensor("vel", [B_PER, H, W, 2], f32, kind="ExternalInput")
    out = nc.dram_tensor("out", [B_PER, H, W, 1], f32, kind="ExternalOutput")
    dmat = nc.dram_tensor("dmat", [P, P], f32, kind="ExternalInput")
    dmat3 = nc.dram_tensor("dmat3", [P, P], f32, kind="ExternalInput")
    emat = nc.dram_tensor("emat", [1, P], f32, kind="ExternalInput")

    # stripe-layout views: [img, partition, stripe, cols]
    state_v = state.ap().rearrange("b (k p) w c -> b p k (w c)", p=P)
    vel_v = vel.ap().rearrange("b (k p) w c -> b p k (w c)", p=P)
    out_v = out.ap().rearrange("b (k p) w c -> b p k (w c)", p=P)

    with tile.TileContext(nc) as tc:
        with ExitStack() as ctx:
            cp = ctx.enter_context(tc.tile_pool(name="cp", bufs=1))
            sp = ctx.enter_context(tc.tile_pool(name="sp", bufs=3))
            vp = ctx.enter_context(tc.tile_pool(name="vp", bufs=3))
            xp = ctx.enter_context(tc.tile_pool(name="xp", bufs=3))
            tp = ctx.enter_context(tc.tile_pool(name="tp", bufs=3))
            pp = ctx.enter_context(tc.tile_pool(name="pp", bufs=2, space="PSUM"))

            D = cp.tile([P, P], f32)
            nc.gpsimd.dma_start(D[:], dmat.ap())
            D3 = cp.tile([P, P], f32)
            nc.gpsimd.dma_start(D3[:], dmat3.ap())
            E = cp.tile([1, P], f32)
            nc.gpsimd.dma_start(E[:], emat.ap())

            def body():
                for i in range(B_PER):
                    s1 = sp.tile([P, FD], f32, name=f"s1_{i}", tag="s1")
                    nc.gpsimd.dma_start(s1[:], state_v[i])
                    v1t = vp.tile([P, 2 * FD], f32, name=f"v1_{i}", tag="v1")
                    nc.scalar.dma_start(v1t[:], vel_v[i])

                    dy_ps = pp.tile([P, FD], f32, name=f"dy{i}", tag="dy")
                    for k in range(3):
                        nc.tensor.matmul(dy_ps[:, k * W:(k + 1) * W], D[:],
                                         s1[:, k * W:(k + 1) * W],
                                         start=True, stop=False)
                    nc.tensor.matmul(dy_ps[:, 3 * W:4 * W], D3[:],
                                     s1[:, 3 * W:4 * W], start=True, stop=True)
                    for k in range(3):
                        nc.tensor.matmul(dy_ps[:, k * W:(k + 1) * W], E[:],
                                         s1[0:1, (k + 1) * W:(k + 2) * W],
                                         start=False, stop=True)

                    dx1 = xp.tile([P, FD], f32, name=f"dx1_{i}", tag="dx1")
                    nc.vector.tensor_sub(dx1[:, 0:FD - 1], s1[:, 1:FD],
                                         s1[:, 0:FD - 1])
                    nc.vector.memset(dx1[:, W - 1::W], 0.0)

                    t1 = tp.tile([P, FD], f32, name=f"t1_{i}", tag="t1")
                    nc.vector.tensor_mul(t1[:], dy_ps[:], v1t[:, 0::2])
                    nc.vector.tensor_mul(dx1[:], dx1[:], v1t[:, 1::2])
                    nc.vector.tensor_add(t1[:], t1[:], dx1[:])
                    nc.sync.dma_start(out_v[i], t1[:])

            if repeats > 1:
                with tc.For_i(0, repeats) as _:
                    body()
            else:
                body()

    nc.compile()
    return nc


def _get_nc():
    if "nc" not in _cache:
        _cache["nc"] = build_nc()
    return _cache["nc"]


def kernel(state_variable: np.ndarray, velocity_field: np.ndarray) -> np.ndarray:
    from concourse.bass_utils import run_bass_kernel_spmd

    nc = _get_nc()
    state_variable = np.ascontiguousarray(state_variable, dtype=np.float32)
    velocity_field = np.ascontiguousarray(velocity_field, dtype=np.float32)
    consts = _consts()
    in_maps = []
    for c in range(N_CORES):
        lo, hi = c * B_PER, (c + 1) * B_PER
        in_maps.append({
            "state": state_variable[lo:hi],
            "vel": velocity_field[lo:hi],
            **consts,
        })
    res = run_bass_kernel_spmd(nc, in_maps, core_ids=list(range(N_CORES)))
    return np.concatenate([r["out"] for r in res.results], axis=0)


# revision 5
# speedup vs baseline: 1.1627x; 1.0023x over previous
"""Trainium2 Bass kernel for nn_Advection (2D advection stencil).

    out[b,i,j] = (s[b,i+1,j]-s[b,i,j])*v[b,i,j,0]
               + (s[b,i,j+1]-s[b,i,j])*v[b,i,j,1]
with symmetric edge padding (forward difference is 0 on the last row/col).

Sharding: pure data parallel — batch 32 split 4-per-core across 8 NeuronCores.

Per-core implementation (memory-bound; steady state measured at ~340 GB/s/core
of HBM traffic ≈ 95% of the ~358 GB/s per-core ceiling):
 - Stripe layout: each 512x512 image lives in SBUF as [128 partitions, 2048];
   partition p, free block k (of 4) holds image row k*128 + p. DRAM reads are
   2KB-contiguous per (partition, block) — full DMA line rate.
 - dy (row shift) runs on the TensorEngine as a banded-difference matmul:
   dy_stripe = D^T @ s_stripe with D = subdiag(+1) + diag(-1), accumulated in
   PSUM; a second K=1 matmul with E (e_0 -> row 127) adds the next stripe's
   first row; the last stripe uses D3 (D with the corner zeroed) so row 511's
   dy is exactly 0. fp32 matmuls keep the result exact to ~5e-8 relative and
   the PE stays off the critical path (DMA-bound kernel).
 - dx (col shift) is a free-dim shifted subtract on the VectorEngine; the
   j=511 junk columns are zeroed by one strided memset.
 - Products and the final add run on the VectorEngine, in-place to save SBUF.
 - DMA rings are kept load/store-pure to avoid HWDGE FIFO head-of-line
   blocking: state loads on the gpsimd SWDGE ring, velocity loads on the
   scalar HWDGE ring, output stores on the sync HWDGE ring. Per-image 1-2MB
   transfers with triple-buffered pools overlap load/compute/store (measured
   faster than coarser 2-image granularity at bufs=2).
"""

import numpy as np

B, H, W = 32, 512, 512
N_CORES = 8
B_PER = B // N_CORES   # 4 images per core
P = 128                # SBUF partitions
KS = H // P            # 4 stripes per image
FD = KS * W            # 2048 free elems per partition per image

_cache = {}


def _consts():
    D = np.zeros((P, P), np.float32)
    for m in range(P):
        D[m, m] = -1.0
        if m + 1 < P:
            D[m + 1, m] = 1.0
    D3 = D.copy()
    D3[P - 1, P - 1] = 0.0
    E = np.zeros((1, P), np.float32)
    E[0, P - 1] = 1.0
    return {"dmat": D, "dmat3": D3, "emat": E}


def build_nc(repeats=1):
    """Build + compile the per-core program. repeats>1 wraps the body in an
    on-device loop (benchmarking only; production uses repeats=1)."""
    from contextlib import ExitStack

    import concourse.tile as tile
    from concourse import bacc, mybir

    f32 = mybir.dt.float32

    nc = bacc.Bacc("TRN2", target_bir_lowering=False)
    state = nc.dram_tensor("state", [B_PER, H, W, 1], f32, kind="ExternalInput")
    vel = nc.dram_tensor("vel", [B_PER, H, W, 2], f32, kind="ExternalInput")
    out = nc.dram_tensor("out", [B_PER, H, W, 1], f32, kind="ExternalOutput")
    dmat = nc.dram_tensor("dmat", [P, P], f32, kind="ExternalInput")
    dmat3 = nc.dram_tensor("dmat3", [P, P], f32, kind="ExternalInput")
    emat = nc.dram_tensor("emat", [1, P], f32, kind="ExternalInput")

    # stripe-layout views: [img, partition, stripe, cols]
    state_v = state.ap().rearrange("b (k p) w c -> b p k (w c)", p=P)
    vel_v = vel.ap().rearrange("b (k p) w c -> b p k (w c)", p=P)
    out_v = out.ap().rearrange("b (k p) w c -> b p k (w c)", p=P)

    with tile.TileContext(nc) as tc:
        with ExitStack() as ctx:
            cp = ctx.enter_context(tc.tile_pool(name="cp", bufs=1))
            sp = ctx.enter_context(tc.tile_pool(name="sp", bufs=3))
            vp = ctx.enter_context(tc.tile_pool(name="vp", bufs=3))
            xp = ctx.enter_context(tc.tile_pool(name="xp", bufs=3))
            tp = ctx.enter_context(tc.tile_pool(name="tp", bufs=3))
            pp = ctx.enter_context(tc.tile_pool(name="pp", bufs=2, space="PSUM"))

            # consts ride the sync ring (idle until the first store) so they
            # never delay the first state load on the SWDGE ring
            D = cp.tile([P, P], f32)
            nc.sync.dma_start(D[:], dmat.ap())
            D3 = cp.tile([P, P], f32)
            nc.sync.dma_start(D3[:], dmat3.ap())
            E = cp.tile([1, P], f32)
            nc.sync.dma_start(E[:], emat.ap())

            # HAM warm-up: ~3.4us of dummy matmuls inside the initial load
            # shadow flips the PE clock gate to 2.4 GHz before real work
            warm = pp.tile([P, W], f32, name="warm", tag="dy")
            for _ in range(32):
                nc.tensor.matmul(warm[:, 0:P], D[:], D[:],
                                 start=True, stop=True)

            def body():
                for i in range(B_PER):
                    s1 = sp.tile([P, FD], f32, name=f"s1_{i}", tag="s1")
                    nc.gpsimd.dma_start(s1[:], state_v[i])
                    v1t = vp.tile([P, 2 * FD], f32, name=f"v1_{i}", tag="v1")
                    nc.scalar.dma_start(v1t[:], vel_v[i])

                    dy_ps = pp.tile([P, FD], f32, name=f"dy{i}", tag="dy")
                    for k in range(3):
                        nc.tensor.matmul(dy_ps[:, k * W:(k + 1) * W], D[:],
                                         s1[:, k * W:(k + 1) * W],
                                         start=True, stop=False)
                    nc.tensor.matmul(dy_ps[:, 3 * W:4 * W], D3[:],
                                     s1[:, 3 * W:4 * W], start=True, stop=True)
                    for k in range(3):
                        nc.tensor.matmul(dy_ps[:, k * W:(k + 1) * W], E[:],
                                         s1[0:1, (k + 1) * W:(k + 2) * W],
                                         start=False, stop=True)

                    dx1 = xp.tile([P, FD], f32, name=f"dx1_{i}", tag="dx1")
                    nc.vector.tensor_sub(dx1[:, 0:FD - 1], s1[:, 1:FD],
                                         s1[:, 0:FD - 1])
                    nc.vector.memset(dx1[:, W - 1::W], 0.0)

                    t1 = tp.tile([P, FD], f32, name=f"t1_{i}", tag="t1")
                    nc.vector.tensor_mul(t1[:], dy_ps[:], v1t[:, 0::2])
                    nc.vector.tensor_mul(dx1[:], dx1[:], v1t[:, 1::2])
                    nc.vector.tensor_add(t1[:], t1[:], dx1[:])
                    nc.sync.dma_start(out_v[i], t1[:])

            if repeats > 1:
                with tc.For_i(0, repeats) as _:
                    body()
            else:
                body()

    nc.compile()
    return nc


def _get_nc():
    if "nc" not in _cache:
        _cache["nc"] = build_nc()
    return _cache["nc"]


def kernel(state_variable: np.ndarray, velocity_field: np.ndarray) -> np.ndarray:
    from concourse.bass_utils import run_bass_kernel_spmd

    nc = _get_nc()
    state_variable = np.ascontiguousarray(state_variable, dtype=np.float32)
    velocity_field = np.ascontiguousarray(velocity_field, dtype=np.float32)
    consts = _consts()
    in_maps = []
    for c in range(N_CORES):
        lo, hi = c * B_PER, (c + 1) * B_PER
        in_maps.append({
            "state": state_variable[lo:hi],
            "vel": velocity_field[lo:hi],
            **consts,
        })
    res = run_bass_kernel_spmd(nc, in_maps, core_ids=list(range(N_CORES)))
    return np.concatenate([r["out"] for r in res.results], axis=0)


# revision 7
# speedup vs baseline: 1.1826x; 1.0172x over previous
"""Trainium2 Bass kernel for nn_Advection (2D advection stencil).

    out[b,i,j] = (s[b,i+1,j]-s[b,i,j])*v[b,i,j,0]
               + (s[b,i,j+1]-s[b,i,j])*v[b,i,j,1]
with symmetric edge padding (forward difference is 0 on the last row/col).

Sharding: pure data parallel — batch 32 split 4-per-core across 8 NeuronCores.

Per-core implementation (memory-bound). Measured via an on-device repeat loop:
~62us per 4-image execution (incl ~5us loop barrier), steady state ~57us vs a
55us floor measured for this exact DMA pattern with zero compute and a 47us
theoretical roofline (16.8 MB/core at 358 GB/s HBM-per-NC):
 - Stripe layout: each 512x512 image lives in SBUF as [128 partitions, 2048];
   partition p, free block k (of 4) holds image row k*128 + p. DRAM reads are
   2KB-contiguous per (partition, block) — full DMA line rate.
 - dy (row shift) runs on the TensorEngine as a banded-difference matmul:
   dy_stripe = D^T @ s_stripe with D = subdiag(+1) + diag(-1), accumulated in
   PSUM; a second K=1 matmul with E (e_0 -> row 127) adds the next stripe's
   first row; the last stripe uses D3 (D with the corner zeroed) so row 511's
   dy is exactly 0. fp32 matmuls keep the result exact to ~5e-8 relative and
   the PE stays off the critical path (DMA-bound kernel).
 - dx (col shift) is a free-dim shifted subtract on the VectorEngine; the
   j=511 junk columns are zeroed by one strided memset.
 - Products and the final add run on the VectorEngine, in-place to save SBUF.
 - DMA rings are kept load/store-pure to avoid HWDGE FIFO head-of-line
   blocking: state loads on the gpsimd SWDGE ring, velocity loads on the
   scalar HWDGE ring, output stores on the sync HWDGE ring. Per-image 1-2MB
   transfers; load pools hold one slot per image (bufs=4) so no load ever
   waits on a slot pinned by an earlier image's compute, work pools are
   triple-buffered (measured faster than coarser 2-image granularity).
"""

import numpy as np

B, H, W = 32, 512, 512
N_CORES = 8
B_PER = B // N_CORES   # 4 images per core
P = 128                # SBUF partitions
KS = H // P            # 4 stripes per image
FD = KS * W            # 2048 free elems per partition per image

_cache = {}


def _consts():
    D = np.zeros((P, P), np.float32)
    for m in range(P):
        D[m, m] = -1.0
        if m + 1 < P:
            D[m + 1, m] = 1.0
    D3 = D.copy()
    D3[P - 1, P - 1] = 0.0
    E = np.zeros((1, P), np.float32)
    E[0, P - 1] = 1.0
    return {"dmat": D, "dmat3": D3, "emat": E}


def build_nc(repeats=1):
    """Build + compile the per-core program. repeats>1 wraps the body in an
    on-device loop (benchmarking only; production uses repeats=1)."""
    from contextlib import ExitStack

    import concourse.tile as tile
    from concourse import bacc, mybir

    f32 = mybir.dt.float32

    nc = bacc.Bacc("TRN2", target_bir_lowering=False)
    state = nc.dram_tensor("state", [B_PER, H, W, 1], f32, kind="ExternalInput")
    vel = nc.dram_tensor("vel", [B_PER, H, W, 2], f32, kind="ExternalInput")
    out = nc.dram_tensor("out", [B_PER, H, W, 1], f32, kind="ExternalOutput")
    dmat = nc.dram_tensor("dmat", [P, P], f32, kind="ExternalInput")
    dmat3 = nc.dram_tensor("dmat3", [P, P], f32, kind="ExternalInput")
    emat = nc.dram_tensor("emat", [1, P], f32, kind="ExternalInput")

    # stripe-layout views: [img, partition, stripe, cols]
    state_v = state.ap().rearrange("b (k p) w c -> b p k (w c)", p=P)
    vel_v = vel.ap().rearrange("b (k p) w c -> b p k (w c)", p=P)
    out_v = out.ap().rearrange("b (k p) w c -> b p k (w c)", p=P)

    with tile.TileContext(nc) as tc:
        with ExitStack() as ctx:
            cp = ctx.enter_context(tc.tile_pool(name="cp", bufs=1))
            # load pools at bufs=4: all four images' loads issue without
            # waiting on a pool slot held by an earlier image's compute
            sp = ctx.enter_context(tc.tile_pool(name="sp", bufs=4))
            vp = ctx.enter_context(tc.tile_pool(name="vp", bufs=4))
            xp = ctx.enter_context(tc.tile_pool(name="xp", bufs=3))
            tp = ctx.enter_context(tc.tile_pool(name="tp", bufs=3))
            pp = ctx.enter_context(tc.tile_pool(name="pp", bufs=2, space="PSUM"))

            # consts ride the sync ring (idle until the first store) so they
            # never delay the first state load on the SWDGE ring
            D = cp.tile([P, P], f32)
            nc.sync.dma_start(D[:], dmat.ap())
            D3 = cp.tile([P, P], f32)
            nc.sync.dma_start(D3[:], dmat3.ap())
            E = cp.tile([1, P], f32)
            nc.sync.dma_start(E[:], emat.ap())

            # HAM warm-up: ~3.4us of dummy matmuls inside the initial load
            # shadow flips the PE clock gate to 2.4 GHz before real work
            warm = pp.tile([P, W], f32, name="warm", tag="dy")
            for _ in range(32):
                nc.tensor.matmul(warm[:, 0:P], D[:], D[:],
                                 start=True, stop=True)

            def body():
                for i in range(B_PER):
                    s1 = sp.tile([P, FD], f32, name=f"s1_{i}", tag="s1")
                    nc.gpsimd.dma_start(s1[:], state_v[i])
                    v1t = vp.tile([P, 2 * FD], f32, name=f"v1_{i}", tag="v1")
                    nc.scalar.dma_start(v1t[:], vel_v[i])

                    dy_ps = pp.tile([P, FD], f32, name=f"dy{i}", tag="dy")
                    for k in range(3):
                        nc.tensor.matmul(dy_ps[:, k * W:(k + 1) * W], D[:],
                                         s1[:, k * W:(k + 1) * W],
                                         start=True, stop=False)
                    nc.tensor.matmul(dy_ps[:, 3 * W:4 * W], D3[:],
                                     s1[:, 3 * W:4 * W], start=True, stop=True)
                    for k in range(3):
                        nc.tensor.matmul(dy_ps[:, k * W:(k + 1) * W], E[:],
                                         s1[0:1, (k + 1) * W:(k + 2) * W],
                                         start=False, stop=True)

                    dx1 = xp.tile([P, FD], f32, name=f"dx1_{i}", tag="dx1")
                    nc.vector.tensor_sub(dx1[:, 0:FD - 1], s1[:, 1:FD],
                                         s1[:, 0:FD - 1])
                    nc.vector.memset(dx1[:, W - 1::W], 0.0)

                    t1 = tp.tile([P, FD], f32, name=f"t1_{i}", tag="t1")
                    nc.vector.tensor_mul(t1[:], dy_ps[:], v1t[:, 0::2])
                    nc.vector.tensor_mul(dx1[:], dx1[:], v1t[:, 1::2])
                    nc.vector.tensor_add(t1[:], t1[:], dx1[:])
                    nc.sync.dma_start(out_v[i], t1[:])

            if repeats > 1:
                with tc.For_i(0, repeats) as _:
                    body()
            else:
                body()

    nc.compile()
    return nc


def _get_nc():
    if "nc" not in _cache:
        _cache["nc"] = build_nc()
    return _cache["nc"]


def kernel(state_variable: np.ndarray, velocity_field: np.ndarray) -> np.ndarray:
    from concourse.bass_utils import run_bass_kernel_spmd

    nc = _get_nc()
    state_variable = np.ascontiguousarray(state_variable, dtype=np.float32)
    velocity_field = np.ascontiguousarray(velocity_field, dtype=np.float32)
    consts = _consts()
    in_maps = []
    for c in range(N_CORES):
        lo, hi = c * B_PER, (c + 1) * B_PER
        in_maps.append({
            "state": state_variable[lo:hi],
            "vel": velocity_field[lo:hi],
            **consts,
        })
    res = run_bass_kernel_spmd(nc, in_maps, core_ids=list(range(N_CORES)))
    return np.concatenate([r["out"] for r in res.results], axis=0)
